# revision 10
# baseline (speedup 1.0000x reference)
"""AdaptiveRankTensorizedLinear (CP, rank 64) forward on 8 TRN2 NeuronCores.

Math: with A = KhatriRao(U1,U2,U3) (4096x64), B = KhatriRao(V1,V2,V3) (4096x64),
    y = (x @ (A * lam)) @ B^T + bias
Data-parallel over the 4096-token batch: each core handles 512 rows of x.
Factors are tiny and replicated; no collectives needed in forward.

Per-core dataflow (all compute on device, bf16 matmuls with f32 accumulate):
  - prologue: build A^T and B^T-augmented (bias as rank-65 row) from the tiny
    factors with broadcast-AP elementwise multiplies (r on partitions), then
    transpose A^T into 32 [128k, 64r] chunks on TensorE.
  - per 128-row m-tile: DMA x f32 -> cast bf16 -> transpose 128x128 chunks on
    TensorE via identity matmuls (k must land on partitions for matmul) ->
    t^T = sum_c A_c^T @ xT_c (PSUM accumulate) -> y = t_aug^T @ BT_aug -> DMA.
"""

import numpy as np

NCORES = 8
B_TOTAL = 4096
B_SHARD = B_TOTAL // NCORES  # 512
IN = 4096
OUT = 4096
D = 16
R = 64

M_TILE = 128
N_MTILES = B_SHARD // M_TILE  # 4
KCHUNK = 128
N_KCHUNKS = IN // KCHUNK  # 32

_CACHE = {}


def _build_nc():
    from contextlib import ExitStack

    from concourse import bacc, mybir
    import concourse.tile as tile
    from concourse.masks import make_identity

    f32 = mybir.dt.float32
    bf16 = mybir.dt.bfloat16

    nc = bacc.Bacc(None, target_bir_lowering=False)

    x_ext = nc.declare_dram_parameter("x", [B_SHARD, IN], f32, isOutput=False)
    U1_ext = nc.declare_dram_parameter("U1", [D, R], f32, isOutput=False)
    U2_ext = nc.declare_dram_parameter("U2", [D, R], f32, isOutput=False)
    U3_ext = nc.declare_dram_parameter("U3", [D, R], f32, isOutput=False)
    V1_ext = nc.declare_dram_parameter("V1", [D, R], f32, isOutput=False)
    V2_ext = nc.declare_dram_parameter("V2", [D, R], f32, isOutput=False)
    V3_ext = nc.declare_dram_parameter("V3", [D, R], f32, isOutput=False)
    lam_ext = nc.declare_dram_parameter("lam", [R], f32, isOutput=False)
    bias_ext = nc.declare_dram_parameter("bias", [OUT], f32, isOutput=False)
    out_ext = nc.declare_dram_parameter("out", [B_SHARD, OUT], f32, isOutput=True)

    with tile.TileContext(nc) as tc, ExitStack() as ctx:
        const = ctx.enter_context(tc.tile_pool(name="const", bufs=1))
        x_pool = ctx.enter_context(tc.tile_pool(name="x", bufs=3))
        xbf_pool = ctx.enter_context(tc.tile_pool(name="xbf", bufs=2))
        xT_pool = ctx.enter_context(tc.tile_pool(name="xT", bufs=4))
        y_pool = ctx.enter_context(tc.tile_pool(name="y", bufs=6))
        psx_pool = ctx.enter_context(tc.tile_pool(name="psx", bufs=3, space="PSUM"))
        pst_pool = ctx.enter_context(tc.tile_pool(name="pst", bufs=2, space="PSUM"))
        psy_pool = ctx.enter_context(tc.tile_pool(name="psy", bufs=3, space="PSUM"))

        # ------------- prologue: tiny factor loads spread across engines -----
        identity = const.tile([128, 128], bf16)
        make_identity(nc, identity[:])

        # transposed factor loads: XiT[r, o] = Xi[o, r]   ([64, 16] each)
        U1T = const.tile([R, D], f32)
        U2T = const.tile([R, D], f32)
        U3T = const.tile([R, D], f32)
        V1T = const.tile([R, D], f32)
        V2T = const.tile([R, D], f32)
        V3T = const.tile([R, D], f32)
        nc.sync.dma_start(out=U1T[:], in_=U1_ext[:].transpose([1, 0]))
        nc.scalar.dma_start(out=U2T[:], in_=U2_ext[:].transpose([1, 0]))
        nc.gpsimd.dma_start(out=U3T[:], in_=U3_ext[:].transpose([1, 0]))
        nc.scalar.dma_start(out=V1T[:], in_=V1_ext[:].transpose([1, 0]))
        nc.sync.dma_start(out=V2T[:], in_=V2_ext[:].transpose([1, 0]))
        nc.gpsimd.dma_start(out=V3T[:], in_=V3_ext[:].transpose([1, 0]))
        lamT = const.tile([R, 1], f32)
        nc.sync.dma_start(out=lamT[:], in_=lam_ext[:].unsqueeze(1))
        bias_sb = const.tile([1, OUT], f32)
        nc.scalar.dma_start(out=bias_sb[:], in_=bias_ext[:].unsqueeze(0))

        # A^T[r, i*256 + j*16 + k] = U1[i,r] * U2[j,r] * U3[k,r]   (bf16)
        W12u = const.tile([R, D * D], f32)
        nc.vector.tensor_mul(
            W12u[:].rearrange("p (a b) -> p a b", a=16),
            U1T[:].unsqueeze(2).broadcast_to([R, D, D]),
            U2T[:].unsqueeze(1).broadcast_to([R, D, D]),
        )
        AT_bf = const.tile([R, IN], bf16)
        nc.vector.tensor_mul(
            AT_bf[:].rearrange("p (w o) -> p w o", o=16),
            W12u[:].unsqueeze(2).broadcast_to([R, D * D, D]),
            U3T[:].unsqueeze(1).broadcast_to([R, D * D, D]),
        )

        # BT_aug rows 0..63: lam[r]*V1[o1,r]*V2[o2,r]*V3[o3,r]; row 64: bias
        V1Ts = const.tile([R, D], f32)
        nc.gpsimd.tensor_mul(V1Ts[:], V1T[:], lamT[:].broadcast_to([R, D]))
        W12v = const.tile([R, D * D], f32)
        nc.gpsimd.tensor_mul(
            W12v[:].rearrange("p (a b) -> p a b", a=16),
            V1Ts[:].unsqueeze(2).broadcast_to([R, D, D]),
            V2T[:].unsqueeze(1).broadcast_to([R, D, D]),
        )
        # big KR expansion split: first half DVE (right after A^T), rest GpSimd
        BT_aug = const.tile([R + 1, OUT], bf16)
        nc.vector.tensor_mul(
            BT_aug[0:R, 0 : OUT // 2].rearrange("p (w o) -> p w o", o=16),
            W12v[:, 0 : D * D // 2].unsqueeze(2).broadcast_to([R, D * D // 2, D]),
            V3T[:].unsqueeze(1).broadcast_to([R, D * D // 2, D]),
        )
        nc.gpsimd.tensor_mul(
            BT_aug[0:R, OUT // 2 : OUT].rearrange("p (w o) -> p w o", o=16),
            W12v[:, D * D // 2 :].unsqueeze(2).broadcast_to([R, D * D // 2, D]),
            V3T[:].unsqueeze(1).broadcast_to([R, D * D // 2, D]),
        )
        nc.scalar.copy(BT_aug[R : R + 1, :], bias_sb[:])

        # A chunks via PE transposes of A^T: A_sb[p, 64c + r] = A[128c + p, r]
        A_sb = const.tile([128, N_KCHUNKS * R], bf16)
        for q in range(4):  # 8 chunks per PSUM bank
            ps_a = psx_pool.tile([128, 512], f32, tag="ps_x")
            for j in range(8):
                c = 8 * q + j
                nc.tensor.matmul(
                    ps_a[:, j * R : (j + 1) * R],
                    AT_bf[:, c * KCHUNK : (c + 1) * KCHUNK],
                    identity[0:R, 0:R],
                    start=True,
                    stop=True,
                )
            nc.vector.tensor_copy(A_sb[:, q * 512 : (q + 1) * 512], ps_a[:])

        # two persistent t_aug tiles (double buffer), ones row preset
        t_aug = []
        for i in range(2):
            t = const.tile([R + 1, M_TILE], bf16, tag=f"t_aug{i}")
            nc.gpsimd.memset(t[R : R + 1, :], 1.0)
            t_aug.append(t)

        # ---------------- main loop ------------------------------------------
        for m in range(N_MTILES):
            x_sb = x_pool.tile([M_TILE, IN], f32)
            # split load so the cast can start at the halfway point
            nc.sync.dma_start(
                out=x_sb[:, 0 : IN // 2],
                in_=x_ext[m * M_TILE : (m + 1) * M_TILE, 0 : IN // 2],
            )
            nc.sync.dma_start(
                out=x_sb[:, IN // 2 :],
                in_=x_ext[m * M_TILE : (m + 1) * M_TILE, IN // 2 :],
            )
            x_bf = xbf_pool.tile([M_TILE, IN], bf16)

            ps_t = pst_pool.tile([R, M_TILE], f32)
            for g in range(N_KCHUNKS // 4):
                lo, hi = g * 4 * KCHUNK, (g + 1) * 4 * KCHUNK
                if g % 2 == 0:
                    nc.scalar.copy(x_bf[:, lo:hi], x_sb[:, lo:hi])
                else:
                    nc.vector.tensor_copy(x_bf[:, lo:hi], x_sb[:, lo:hi])
                ps_x = psx_pool.tile([128, 4 * KCHUNK], f32)
                for j in range(4):
                    c = 4 * g + j
                    nc.tensor.matmul(
                        ps_x[:, j * KCHUNK : (j + 1) * KCHUNK],
                        x_bf[:, c * KCHUNK : (c + 1) * KCHUNK],
                        identity[:],
                        start=True,
                        stop=True,
                    )
                xT_sb = xT_pool.tile([128, 4 * KCHUNK], bf16)
                if g % 2 == 0:
                    nc.vector.tensor_copy(xT_sb[:], ps_x[:])
                else:
                    nc.scalar.copy(xT_sb[:], ps_x[:])
                for j in range(4):
                    c = 4 * g + j
                    nc.tensor.matmul(
                        ps_t[:],
                        A_sb[:, c * R : (c + 1) * R],
                        xT_sb[:, j * KCHUNK : (j + 1) * KCHUNK],
                        start=(c == 0),
                        stop=(c == N_KCHUNKS - 1),
                    )

            tt = t_aug[m % 2]
            nc.vector.tensor_copy(tt[0:R, :], ps_t[:])

            for n in range(8):
                ps_y = psy_pool.tile([M_TILE, 512], f32)
                nc.tensor.matmul(
                    ps_y[:],
                    tt[:],
                    BT_aug[:, n * 512 : (n + 1) * 512],
                    start=True,
                    stop=True,
                )
                y_sb = y_pool.tile([M_TILE, 512], f32)
                if n % 2 == 0:
                    nc.scalar.copy(y_sb[:], ps_y[:])
                else:
                    nc.vector.tensor_copy(y_sb[:], ps_y[:])
                nc.sync.dma_start(
                    out=out_ext[
                        m * M_TILE : (m + 1) * M_TILE, n * 512 : (n + 1) * 512
                    ],
                    in_=y_sb[:],
                )

    nc.compile()
    return nc


def _get_nc():
    if "nc" not in _CACHE:
        _CACHE["nc"] = _build_nc()
    return _CACHE["nc"]


def kernel(x, U1, U2, U3, V1, V2, V3, lam, bias):
    from concourse.bass_utils import run_bass_kernel_spmd

    nc = _get_nc()

    x = np.ascontiguousarray(np.asarray(x, dtype=np.float32))
    small = {
        "U1": U1, "U2": U2, "U3": U3,
        "V1": V1, "V2": V2, "V3": V3,
        "lam": lam, "bias": bias,
    }
    small = {
        k: np.ascontiguousarray(np.asarray(v, dtype=np.float32))
        for k, v in small.items()
    }

    in_maps = [
        {"x": x[i * B_SHARD : (i + 1) * B_SHARD], **small} for i in range(NCORES)
    ]
    res = run_bass_kernel_spmd(nc, in_maps, core_ids=list(range(NCORES)))
    _CACHE["last_results"] = res
    out = np.concatenate(
        [np.asarray(res.results[i]["out"]) for i in range(NCORES)], axis=0
    )
    return out.astype(np.float32)


def last_exec_time_ns():
    res = _CACHE.get("last_results")
    return None if res is None else res.exec_time_ns
